# revision 12
# baseline (speedup 1.0000x reference)
"""Multi-head attention Trainium2 Bass kernel — v2.

Problem: B=2, S=2048, D=1024, H=16 heads, DH=64, causal (or arbitrary) mask.
Sharding: 8 cores = data-parallel over B (2) x tensor-parallel over head
groups (4 groups of 4 heads). Each core computes QKV projections for its
head group, attention for its 4 heads, and a partial output projection
(attended @ Wo-shard). Host sums the 4 partials per batch and adds bo.

v2 changes vs v1:
  - x is uploaded pre-transposed (xT [D, S]) so the kernel does no PE
    transposes and no transpose-evacuation copies.
  - bf16 everywhere on the input side (x, weights, scores operands, exp
    values, v, attT); PSUM accumulation stays fp32.
  - Causal masking without the [0:off] zero-kill: score matmuls and exp
    are restricted to the live range, the in-block triangle is masked by
    one [128,128] DVE multiply, and PV matmuls are range-restricted so
    dead columns are simply never accumulated (split stop flags so each
    acc region gets its stop from its true last contributor).
  - Per-s-block write-once tiles (qT/kT/v/attT) so next-block projection
    writes never WAR-serialize against current-block attention reads.
  - Emission order = scheduler priority: attention(sb) leads; proj(sb+1)
    and outproj(sb-1) emitted after it act as PE filler during the
    Act-bound softmax stretch.
"""
import numpy as np

B, S, D = 2, 2048, 1024
H, DH = 16, 64
NCORES = 8
HPC = 4              # heads per core
DIMS = HPC * DH      # 256 projection dims per core
NSB = S // 512       # 4 q/s blocks of 512
NST = S // 128       # 16 s tiles of 128
NDC = D // 128       # 8 contraction chunks
NCST = 644           # consts: [0:128] triu ones | [128:644] ones

_PROG_CACHE = {}


def build_program(mode="causal", has_bias=False, reps=1, phases=None):
    import concourse.bacc as bacc
    import concourse.mybir as mybir
    import concourse.tile as tile

    DT = mybir.dt.float32
    FR = mybir.dt.float32r
    BF = mybir.dt.bfloat16
    Act = mybir.ActivationFunctionType
    MUL = mybir.AluOpType.mult

    nc = bacc.Bacc("TRN2", target_bir_lowering=False, debug=False,
                   num_devices=NCORES)

    xt = nc.dram_tensor("xt", [D, S], BF, kind="ExternalInput")
    wq = nc.dram_tensor("wq", [D, DIMS], BF, kind="ExternalInput")
    wk = nc.dram_tensor("wk", [D, DIMS], BF, kind="ExternalInput")
    wv = nc.dram_tensor("wv", [D, DIMS], BF, kind="ExternalInput")
    wo = nc.dram_tensor("wo", [DIMS, D], BF, kind="ExternalInput")
    cin = nc.dram_tensor("cin", [128, NCST], BF, kind="ExternalInput")
    if has_bias:
        wqb_d = nc.dram_tensor("wqb", [1, DIMS], BF, kind="ExternalInput")
        wvb_d = nc.dram_tensor("wvb", [1, DIMS], BF, kind="ExternalInput")
    if mode == "general":
        mtin = nc.dram_tensor("maskt", [S, S], BF, kind="ExternalInput")
    outp = nc.dram_tensor("outp", [S, D], DT, kind="ExternalOutput")

    causal = mode == "causal"

    with tile.TileContext(nc) as tc:
        with (
            tc.tile_pool(name="pw", bufs=1) as pw,
            tc.tile_pool(name="pact", bufs=2) as pact,
            tc.tile_pool(name="pexp", bufs=6) as pexp,
            tc.tile_pool(name="pmask", bufs=4) as pmask,
            tc.tile_pool(name="pout", bufs=3) as pout,
            tc.tile_pool(name="pmisc", bufs=4) as pmisc,
            tc.tile_pool(name="psS", bufs=2, space="PSUM") as psS,   # scores, 2 banks each
            tc.tile_pool(name="psA", bufs=2, space="PSUM") as psA,   # PV accumulators
            tc.tile_pool(name="psP", bufs=2, space="PSUM") as psP,   # proj/outproj
        ):
            def body(bi=0):
                # ---- weights + consts + xT ----
                # DMA order = first-use order: consts + wq/wk + x(sb=0)
                # first so proj(0) starts as early as possible; wot last
                # (first used by outproj(0), well into the kernel).
                wqt = pw.tile([128, NDC, DIMS], BF, tag="wqt")
                wkt = pw.tile([128, NDC, DIMS], BF, tag="wkt")
                wvt = pw.tile([128, NDC, DIMS], BF, tag="wvt")
                wot = pw.tile([128, 2, D], BF, tag="wot")
                cst = pw.tile([128, NCST], BF, tag="cst")
                xts = [pw.tile([128, NDC, 512], BF, tag=f"xts{sb}", bufs=2,
                               name=f"xts{sb}_{bi}") for sb in range(NSB)]
                # one whole-tensor DMA each (3D access patterns): same
                # bytes, ~6x fewer DMA instructions and semaphore updates
                nc.sync.dma_start(cst[:], cin[:])
                nc.sync.dma_start(
                    wqt[:], wq[:].rearrange("(d p) n -> p d n", p=128))
                nc.sync.dma_start(
                    xts[0][:],
                    xt[:, 0:512].rearrange("(d p) c -> p d c", p=128))
                nc.sync.dma_start(
                    wkt[:], wk[:].rearrange("(d p) n -> p d n", p=128))
                nc.sync.dma_start(
                    wvt[:], wv[:].rearrange("(d p) n -> p d n", p=128))
                for sb in range(1, NSB):
                    nc.sync.dma_start(
                        xts[sb][:],
                        xt[:, sb * 512:(sb + 1) * 512].rearrange(
                            "(d p) c -> p d c", p=128))
                nc.sync.dma_start(
                    wot[:], wo[:].rearrange("(t p) n -> p t n", p=128))
                tri = cst[:, 0:128]
                ones = cst[:, 128:NCST]
                if has_bias:
                    wqb = pw.tile([1, DIMS], BF, tag="wqb")
                    wvb = pw.tile([1, DIMS], BF, tag="wvb")
                    nc.sync.dma_start(wqb[:], wqb_d[:])
                    nc.sync.dma_start(wvb[:], wvb_d[:])

                qT = [pact.tile([128, 2, 512], BF, tag=f"qT{sb}",
                                name=f"qT{sb}_{bi}") for sb in range(NSB)]
                kT = [pact.tile([128, 2, 512], BF, tag=f"kT{sb}",
                                name=f"kT{sb}_{bi}") for sb in range(NSB)]
                vA = [pact.tile([128, 4, HPC, 65], BF, tag=f"v{sb}",
                                name=f"v{sb}_{bi}") for sb in range(NSB)]
                # per-(sb,pair) so outproj's t=0 accumulation can start
                # while pair 1's normalization is still in flight
                attT = [[pact.tile([128, 512], BF, tag=f"aT{sb}_{p}",
                                   name=f"aT{sb}_{p}_{bi}") for p in range(2)]
                        for sb in range(NSB)]

                def emit_proj(sb):
                    # qT / kT: [dims-in-t, q] per t-tile of 128 dims
                    for wname, w3, dstT in (("q", wqt, qT[sb]),
                                            ("k", wkt, kT[sb])):
                        for t in range(2):
                            ps = psP.tile([128, 512], DT, tag="pp")
                            qbias = has_bias and wname == "q"
                            for d in range(NDC):
                                nc.tensor.matmul(
                                    ps[:], w3[:, d, t * 128:(t + 1) * 128],
                                    xts[sb][:, d, :],
                                    start=(d == 0),
                                    stop=(d == NDC - 1 and not qbias))
                            if qbias:
                                nc.tensor.matmul(
                                    ps[:], wqb[:, t * 128:(t + 1) * 128],
                                    ones[0:1, 0:512], start=False, stop=True)
                            nc.vector.tensor_copy(dstT[:, t, :], ps[:])
                    # v natural layout [s, h, dh] + ones column
                    for stl in range(4):
                        ps = psP.tile([128, DIMS], DT, tag="pp")
                        for d in range(NDC):
                            nc.tensor.matmul(
                                ps[:], xts[sb][:, d, stl * 128:(stl + 1) * 128],
                                wvt[:, d, :],
                                start=(d == 0),
                                stop=(d == NDC - 1 and not has_bias))
                        if has_bias:
                            nc.tensor.matmul(ps[:], ones[0:1, 0:128], wvb[:],
                                             start=False, stop=True)
                        nc.vector.tensor_copy(
                            vA[sb][:, stl, :, 0:64],
                            ps[:].rearrange("p (h e) -> p h e", h=HPC))
                        nc.gpsimd.tensor_copy(vA[sb][:, stl, :, 64],
                                              ones[:, 0:4])

                def emit_attn(qb):
                    nkv = 4 * qb + 4 if causal else NST
                    ncg = nkv // 2
                    q0 = qb * 512
                    for pair in range(2):
                        accs = [psA.tile([65, 512], DT, tag="acc",
                                         name=f"acc{qb}_{pair}_{i}_{bi}")
                                for i in range(2)]
                        # PV lags scores/exp by LAG groups; shallow keeps
                        # in-flight concurrency (and HW SBUF port contention)
                        # low
                        LAG = 2
                        exp_q = {}
                        for u in range(ncg + LAG):
                            if u < ncg:
                                cg = u
                                if mode == "general":
                                    mt = pmask.tile([128, 2, 512], BF, tag="mt")
                                    for j in range(2):
                                        c = 2 * cg + j
                                        nc.sync.dma_start(
                                            mt[:, j, :],
                                            mtin[c * 128:(c + 1) * 128,
                                                 q0:q0 + 512])
                                for h2 in range(2):
                                    lo, hi = h2 * 64, (h2 + 1) * 64
                                    scps = psS.tile([128, 2, 512], DT, tag="sc")
                                    offs = []
                                    for j in range(2):
                                        c = 2 * cg + j
                                        off = (max(0, c * 128 - q0)
                                               if causal else 0)
                                        offs.append(off)
                                        nc.tensor.matmul(
                                            scps[:, j, off:512],
                                            kT[c // 4][lo:hi, pair,
                                                       (c % 4) * 128:
                                                       (c % 4) * 128 + 128],
                                            qT[qb][lo:hi, pair, off:512],
                                            start=True, stop=True)
                                    expt = pexp.tile([128, 2, 512], BF,
                                                     tag="exp")
                                    scl = float(1.0 / np.sqrt(DH))
                                    # one full-tile exp beats two restricted
                                    # ones unless the dead range is large;
                                    # junk cols feed exp but are never read
                                    # by the restricted PV.
                                    if offs[1] <= 128:
                                        # full-tile exp; the junk in
                                        # j=1 [0:128] is zero-killed by the
                                        # Pool memset below so PV can run
                                        # full-width
                                        nc.scalar.activation(
                                            expt[:], scps[:], Act.Exp,
                                            scale=scl)
                                        if causal and offs[1] == 128:
                                            nc.gpsimd.memset(
                                                expt[:, 1, 0:128], 0.0)
                                    else:
                                        for j in range(2):
                                            o = offs[j]
                                            # dead region never exp'd: zero
                                            # it (independent range, Pool
                                            # runs it early)
                                            nc.gpsimd.memset(
                                                expt[:, j, 0:o], 0.0)
                                            nc.scalar.activation(
                                                expt[:, j, o:512],
                                                scps[:, j, o:512], Act.Exp,
                                                scale=scl)
                                    for j in range(2):
                                        c = 2 * cg + j
                                        if causal and c * 128 >= q0:
                                            o = offs[j]
                                            nc.vector.tensor_tensor(
                                                expt[:, j, o:o + 128],
                                                expt[:, j, o:o + 128],
                                                tri[:], MUL)
                                        elif mode == "general":
                                            nc.vector.tensor_tensor(
                                                expt[:, j, :], expt[:, j, :],
                                                mt[:, j, :], MUL)
                                    exp_q[(cg, h2)] = (expt, offs)
                            if u >= LAG:
                                cg = u - LAG
                                for h2 in range(2):
                                    expt, offs = exp_q.pop((cg, h2))
                                    h = pair * 2 + h2
                                    for j in range(2):
                                        c = 2 * cg + j
                                        vl = vA[c // 4][:, c % 4, h, :]
                                        nc.tensor.matmul(
                                            accs[h2][0:65, :], vl,
                                            expt[:, j, :],
                                            start=(c == 0),
                                            stop=(c == nkv - 1))
                        for h2 in range(2):
                            # bit-exact reciprocal is ~6 cpe and serial on
                            # this single-partition row (~3.4us on HW); the
                            # NR-seeded approximation is ~5x faster and its
                            # ~18 correct bits dwarf the bf16 output anyway
                            dnr = pmisc.tile([1, 512], DT, tag="dnr")
                            nc.vector.tensor_copy(dnr[:],
                                                  accs[h2][64:65, :])
                            recip = pmisc.tile([1, 512], DT, tag="recip")
                            nc.vector.reciprocal_approx_fast(
                                recip[:], dnr[:])
                            recipb = pmisc.tile([64, 512], DT, tag="recipb")
                            nc.gpsimd.partition_broadcast(recipb[:],
                                                          recip[:])
                            nc.vector.tensor_tensor(
                                attT[qb][pair][h2 * 64:(h2 + 1) * 64, :],
                                accs[h2][0:64, :], recipb[:], MUL)

                def emit_outproj(sb):
                    # last-emitted block's copies split DVE/Act: nothing
                    # else left to overlap, so don't serialize on one engine
                    tail = sb == NSB - 1
                    for stl in range(4):
                        st = sb * 4 + stl
                        ot = pout.tile([128, D], DT, tag="out")
                        for half in range(2):
                            ps = psP.tile([128, 512], DT, tag="pp")
                            for t in range(2):
                                nc.tensor.matmul(
                                    ps[:],
                                    attT[sb][t][:, stl * 128:(stl + 1) * 128],
                                    wot[:, t, half * 512:(half + 1) * 512],
                                    start=(t == 0), stop=(t == 1))
                            dst = ot[:, half * 512:(half + 1) * 512]
                            if tail and half == 1:
                                nc.scalar.activation(dst, ps[:], Act.Copy)
                            else:
                                nc.vector.tensor_copy(dst, ps[:])
                        nc.sync.dma_start(outp[st * 128:(st + 1) * 128, :],
                                          ot[:])

                # Emission order = scheduler priority: each attn leads;
                # proj(sb+1)/outproj(sb-1) emitted after it act as PE filler
                # during the Act-bound softmax stretch. Only valid for
                # causal, where attn(sb) reads K/V blocks <= sb; with a
                # full mask every attn reads ALL K/V blocks, so all
                # projections must be emitted first.
                if causal:
                    emit_proj(0)
                    for sb in range(NSB):
                        emit_attn(sb)
                        if sb + 1 < NSB:
                            emit_proj(sb + 1)
                        if sb >= 1:
                            emit_outproj(sb - 1)
                    emit_outproj(NSB - 1)
                else:
                    for sb in range(NSB):
                        emit_proj(sb)
                    for sb in range(NSB):
                        emit_attn(sb)
                        if sb >= 1:
                            emit_outproj(sb - 1)
                    emit_outproj(NSB - 1)

            UNROLL = 8
            if reps == 1:
                body()
            elif reps < 0:
                # debug: -reps serial bodies, no loop (TimelineSim-able)
                for bi in range(-reps):
                    body(bi)
            elif reps % UNROLL == 0:
                # unrolled: tile tags are shared across the two bodies
                # (natural WAR pipelining) while names stay unique; halves
                # the per-iteration drain + all-engine barrier cost
                with tc.For_i(0, reps // UNROLL, 1):
                    for bi in range(UNROLL):
                        body(bi)
            else:
                with tc.For_i(0, reps, 1):
                    body()

    nc.compile()
    return nc


def _bf16(a):
    import ml_dtypes
    return np.ascontiguousarray(np.asarray(a, np.float32).astype(
        ml_dtypes.bfloat16))


def _consts_array():
    import ml_dtypes
    c = np.zeros((128, NCST), dtype=np.float32)
    c[:, 0:128] = np.triu(np.ones((128, 128), np.float32))
    c[:, 128:NCST] = 1.0
    return c.astype(ml_dtypes.bfloat16)


def make_in_maps(x, mask, Wq, bq, Wk, bk, Wv, bv, Wo, bo):
    x = np.asarray(x, np.float32)
    m = np.asarray(mask)[0, 0]
    mb = (m != 0)
    if mb.all():
        mode = "none"
    elif np.array_equal(mb, np.tril(np.ones((S, S), bool))):
        mode = "causal"
    else:
        mode = "general"
    # bk shifts all scores for a given query equally -> softmax-invariant;
    # only bq and bv affect the output.
    has_bias = bool(np.any(bq) or np.any(bv))

    Wq = np.asarray(Wq, np.float32)
    Wk = np.asarray(Wk, np.float32)
    Wv = np.asarray(Wv, np.float32)
    Wo = np.asarray(Wo, np.float32)
    consts = _consts_array()
    maskt = _bf16(mb.T.astype(np.float32)) if mode == "general" else None

    in_maps = []
    for c in range(NCORES):
        b, hg = divmod(c, HPC)
        cols = slice(hg * DIMS, (hg + 1) * DIMS)
        im = {
            "xt": _bf16(x[b].T),
            "wq": _bf16(Wq[:, cols]),
            "wk": _bf16(Wk[:, cols]),
            "wv": _bf16(Wv[:, cols]),
            "wo": _bf16(Wo[hg * DIMS:(hg + 1) * DIMS, :]),
            "cin": consts,
        }
        if has_bias:
            im["wqb"] = _bf16(np.asarray(bq, np.float32)[None, cols])
            im["wvb"] = _bf16(np.asarray(bv, np.float32)[None, cols])
        if maskt is not None:
            im["maskt"] = maskt
        in_maps.append(im)
    return in_maps, mode, has_bias


def gather_output(results, bo):
    out = np.zeros((B, S, D), dtype=np.float32)
    for c in range(NCORES):
        out[c // HPC] += results[c]["outp"]
    out += np.asarray(bo, np.float32)[None, None, :]
    return out


def run(in_maps, mode, has_bias, reps=1, phases=None):
    from concourse.bass_utils import run_bass_kernel_spmd
    key = (mode, has_bias, reps)
    if key not in _PROG_CACHE:
        _PROG_CACHE[key] = build_program(mode, has_bias, reps)
    nc = _PROG_CACHE[key]
    return run_bass_kernel_spmd(nc, in_maps, core_ids=list(range(NCORES)))


def kernel(x, mask, Wq, bq, Wk, bk, Wv, bv, Wo, bo):
    in_maps, mode, has_bias = make_in_maps(x, mask, Wq, bq, Wk, bk, Wv, bv,
                                           Wo, bo)
    r = run(in_maps, mode, has_bias, reps=1)
    return gather_output(r.results, bo)



# revision 17
# speedup vs baseline: 1.0171x; 1.0171x over previous
"""Multi-head attention Trainium2 Bass kernel — v2.

Problem: B=2, S=2048, D=1024, H=16 heads, DH=64, causal (or arbitrary) mask.
Sharding: 8 cores = data-parallel over B (2) x tensor-parallel over head
groups (4 groups of 4 heads). Each core computes QKV projections for its
head group, attention for its 4 heads, and a partial output projection
(attended @ Wo-shard). Host sums the 4 partials per batch and adds bo.

v2 changes vs v1:
  - x is uploaded pre-transposed (xT [D, S]) so the kernel does no PE
    transposes and no transpose-evacuation copies.
  - bf16 everywhere on the input side (x, weights, scores operands, exp
    values, v, attT); PSUM accumulation stays fp32.
  - Causal masking without the [0:off] zero-kill: score matmuls and exp
    are restricted to the live range, the in-block triangle is masked by
    one [128,128] DVE multiply, and PV matmuls are range-restricted so
    dead columns are simply never accumulated (split stop flags so each
    acc region gets its stop from its true last contributor).
  - Per-s-block write-once tiles (qT/kT/v/attT) so next-block projection
    writes never WAR-serialize against current-block attention reads.
  - Emission order = scheduler priority: attention(sb) leads; proj(sb+1)
    and outproj(sb-1) emitted after it act as PE filler during the
    Act-bound softmax stretch.
"""
import numpy as np

B, S, D = 2, 2048, 1024
H, DH = 16, 64
NCORES = 8
HPC = 4              # heads per core
DIMS = HPC * DH      # 256 projection dims per core
NSB = S // 512       # 4 q/s blocks of 512
NST = S // 128       # 16 s tiles of 128
NDC = D // 128       # 8 contraction chunks
NCST = 644           # consts: [0:128] triu ones | [128:644] ones

_PROG_CACHE = {}


def build_program(mode="causal", has_bias=False, reps=1, phases=None):
    import concourse.bacc as bacc
    import concourse.mybir as mybir
    import concourse.tile as tile

    DT = mybir.dt.float32
    FR = mybir.dt.float32r
    BF = mybir.dt.bfloat16
    Act = mybir.ActivationFunctionType
    MUL = mybir.AluOpType.mult

    nc = bacc.Bacc("TRN2", target_bir_lowering=False, debug=False,
                   num_devices=NCORES)

    xt = nc.dram_tensor("xt", [D, S], BF, kind="ExternalInput")
    wq = nc.dram_tensor("wq", [D, DIMS], BF, kind="ExternalInput")
    wk = nc.dram_tensor("wk", [D, DIMS], BF, kind="ExternalInput")
    wv = nc.dram_tensor("wv", [D, DIMS], BF, kind="ExternalInput")
    wo = nc.dram_tensor("wo", [DIMS, D], BF, kind="ExternalInput")
    cin = nc.dram_tensor("cin", [128, NCST], BF, kind="ExternalInput")
    if has_bias:
        wqb_d = nc.dram_tensor("wqb", [1, DIMS], BF, kind="ExternalInput")
        wvb_d = nc.dram_tensor("wvb", [1, DIMS], BF, kind="ExternalInput")
    if mode == "general":
        mtin = nc.dram_tensor("maskt", [S, S], BF, kind="ExternalInput")
    outp = nc.dram_tensor("outp", [S, D], BF, kind="ExternalOutput")

    causal = mode == "causal"

    with tile.TileContext(nc) as tc:
        with (
            tc.tile_pool(name="pw", bufs=1) as pw,
            tc.tile_pool(name="pact", bufs=2) as pact,
            tc.tile_pool(name="pexp", bufs=6) as pexp,
            tc.tile_pool(name="pmask", bufs=4) as pmask,
            tc.tile_pool(name="pout", bufs=3) as pout,
            tc.tile_pool(name="pmisc", bufs=4) as pmisc,
            tc.tile_pool(name="psS", bufs=2, space="PSUM") as psS,   # scores, 2 banks each
            tc.tile_pool(name="psA", bufs=2, space="PSUM") as psA,   # PV accumulators
            tc.tile_pool(name="psP", bufs=2, space="PSUM") as psP,   # proj/outproj
        ):
            def body(bi=0):
                # ---- weights + consts + xT ----
                # DMA order = first-use order: consts + wq/wk + x(sb=0)
                # first so proj(0) starts as early as possible; wot last
                # (first used by outproj(0), well into the kernel).
                wqt = pw.tile([128, NDC, DIMS], BF, tag="wqt")
                wkt = pw.tile([128, NDC, DIMS], BF, tag="wkt")
                wvt = pw.tile([128, NDC, DIMS], BF, tag="wvt")
                wot = pw.tile([128, 2, D], BF, tag="wot")
                cst = pw.tile([128, NCST], BF, tag="cst")
                xts = [pw.tile([128, NDC, 512], BF, tag=f"xts{sb}", bufs=2,
                               name=f"xts{sb}_{bi}") for sb in range(NSB)]
                # one whole-tensor DMA each (3D access patterns): same
                # bytes, ~6x fewer DMA instructions and semaphore updates
                nc.sync.dma_start(cst[:], cin[:])
                nc.sync.dma_start(
                    wqt[:], wq[:].rearrange("(d p) n -> p d n", p=128))
                nc.sync.dma_start(
                    xts[0][:],
                    xt[:, 0:512].rearrange("(d p) c -> p d c", p=128))
                nc.sync.dma_start(
                    wkt[:], wk[:].rearrange("(d p) n -> p d n", p=128))
                nc.sync.dma_start(
                    wvt[:], wv[:].rearrange("(d p) n -> p d n", p=128))
                for sb in range(1, NSB):
                    nc.sync.dma_start(
                        xts[sb][:],
                        xt[:, sb * 512:(sb + 1) * 512].rearrange(
                            "(d p) c -> p d c", p=128))
                nc.sync.dma_start(
                    wot[:], wo[:].rearrange("(t p) n -> p t n", p=128))
                tri = cst[:, 0:128]
                ones = cst[:, 128:NCST]
                if has_bias:
                    wqb = pw.tile([1, DIMS], BF, tag="wqb")
                    wvb = pw.tile([1, DIMS], BF, tag="wvb")
                    nc.sync.dma_start(wqb[:], wqb_d[:])
                    nc.sync.dma_start(wvb[:], wvb_d[:])

                qT = [pact.tile([128, 2, 512], BF, tag=f"qT{sb}",
                                name=f"qT{sb}_{bi}") for sb in range(NSB)]
                kT = [pact.tile([128, 2, 512], BF, tag=f"kT{sb}",
                                name=f"kT{sb}_{bi}") for sb in range(NSB)]
                vA = [pact.tile([128, 4, HPC, 65], BF, tag=f"v{sb}",
                                name=f"v{sb}_{bi}") for sb in range(NSB)]
                # per-(sb,pair) so outproj's t=0 accumulation can start
                # while pair 1's normalization is still in flight
                attT = [[pact.tile([128, 512], BF, tag=f"aT{sb}_{p}",
                                   name=f"aT{sb}_{p}_{bi}") for p in range(2)]
                        for sb in range(NSB)]

                def emit_proj(sb):
                    # qT / kT: [dims-in-t, q] per t-tile of 128 dims
                    for wname, w3, dstT in (("q", wqt, qT[sb]),
                                            ("k", wkt, kT[sb])):
                        for t in range(2):
                            ps = psP.tile([128, 512], DT, tag="pp")
                            qbias = has_bias and wname == "q"
                            for d in range(NDC):
                                nc.tensor.matmul(
                                    ps[:], w3[:, d, t * 128:(t + 1) * 128],
                                    xts[sb][:, d, :],
                                    start=(d == 0),
                                    stop=(d == NDC - 1 and not qbias))
                            if qbias:
                                nc.tensor.matmul(
                                    ps[:], wqb[:, t * 128:(t + 1) * 128],
                                    ones[0:1, 0:512], start=False, stop=True)
                            nc.vector.tensor_copy(dstT[:, t, :], ps[:])
                    # v natural layout [s, h, dh] + ones column
                    for stl in range(4):
                        ps = psP.tile([128, DIMS], DT, tag="pp")
                        for d in range(NDC):
                            nc.tensor.matmul(
                                ps[:], xts[sb][:, d, stl * 128:(stl + 1) * 128],
                                wvt[:, d, :],
                                start=(d == 0),
                                stop=(d == NDC - 1 and not has_bias))
                        if has_bias:
                            nc.tensor.matmul(ps[:], ones[0:1, 0:128], wvb[:],
                                             start=False, stop=True)
                        nc.vector.tensor_copy(
                            vA[sb][:, stl, :, 0:64],
                            ps[:].rearrange("p (h e) -> p h e", h=HPC))
                        nc.gpsimd.tensor_copy(vA[sb][:, stl, :, 64],
                                              ones[:, 0:4])

                def emit_attn(qb):
                    nkv = 4 * qb + 4 if causal else NST
                    ncg = nkv // 2
                    q0 = qb * 512
                    for pair in range(2):
                        accs = [psA.tile([65, 512], DT, tag="acc",
                                         name=f"acc{qb}_{pair}_{i}_{bi}")
                                for i in range(2)]
                        # PV lags scores/exp by LAG groups; shallow keeps
                        # in-flight concurrency (and HW SBUF port contention)
                        # low
                        LAG = 2
                        exp_q = {}
                        for u in range(ncg + LAG):
                            if u < ncg:
                                cg = u
                                if mode == "general":
                                    mt = pmask.tile([128, 2, 512], BF, tag="mt")
                                    for j in range(2):
                                        c = 2 * cg + j
                                        nc.sync.dma_start(
                                            mt[:, j, :],
                                            mtin[c * 128:(c + 1) * 128,
                                                 q0:q0 + 512])
                                for h2 in range(2):
                                    lo, hi = h2 * 64, (h2 + 1) * 64
                                    scps = psS.tile([128, 2, 512], DT, tag="sc")
                                    offs = []
                                    for j in range(2):
                                        c = 2 * cg + j
                                        off = (max(0, c * 128 - q0)
                                               if causal else 0)
                                        offs.append(off)
                                        nc.tensor.matmul(
                                            scps[:, j, off:512],
                                            kT[c // 4][lo:hi, pair,
                                                       (c % 4) * 128:
                                                       (c % 4) * 128 + 128],
                                            qT[qb][lo:hi, pair, off:512],
                                            start=True, stop=True)
                                    expt = pexp.tile([128, 2, 512], BF,
                                                     tag="exp")
                                    scl = float(1.0 / np.sqrt(DH))
                                    # one full-tile exp beats two restricted
                                    # ones unless the dead range is large;
                                    # junk cols feed exp but are never read
                                    # by the restricted PV.
                                    if offs[1] <= 128:
                                        nc.scalar.activation(
                                            expt[:], scps[:], Act.Exp,
                                            scale=scl)
                                    else:
                                        for j in range(2):
                                            o = offs[j]
                                            nc.scalar.activation(
                                                expt[:, j, o:512],
                                                scps[:, j, o:512], Act.Exp,
                                                scale=scl)
                                    for j in range(2):
                                        c = 2 * cg + j
                                        if causal and c * 128 >= q0:
                                            o = offs[j]
                                            nc.vector.tensor_tensor(
                                                expt[:, j, o:o + 128],
                                                expt[:, j, o:o + 128],
                                                tri[:], MUL)
                                        elif mode == "general":
                                            nc.vector.tensor_tensor(
                                                expt[:, j, :], expt[:, j, :],
                                                mt[:, j, :], MUL)
                                    exp_q[(cg, h2)] = (expt, offs)
                            if u >= LAG:
                                cg = u - LAG
                                for h2 in range(2):
                                    expt, offs = exp_q.pop((cg, h2))
                                    h = pair * 2 + h2
                                    for j in range(2):
                                        c = 2 * cg + j
                                        vl = vA[c // 4][:, c % 4, h, :]
                                        if causal and c * 128 >= q0:
                                            o = offs[j]
                                            nc.tensor.matmul(
                                                accs[h2][0:65, o:o + 128],
                                                vl, expt[:, j, o:o + 128],
                                                start=(c == 0), stop=True)
                                            if o + 128 < 512:
                                                nc.tensor.matmul(
                                                    accs[h2][0:65, o + 128:512],
                                                    vl,
                                                    expt[:, j, o + 128:512],
                                                    start=(c == 0),
                                                    stop=False)
                                        else:
                                            nc.tensor.matmul(
                                                accs[h2][0:65, :], vl,
                                                expt[:, j, :],
                                                start=(c == 0),
                                                stop=(not causal
                                                      and c == nkv - 1))
                        for h2 in range(2):
                            # bit-exact reciprocal is ~6 cpe and serial on
                            # this single-partition row (~3.4us on HW); the
                            # NR-seeded approximation is ~5x faster and its
                            # ~18 correct bits dwarf the bf16 output anyway
                            dnr = pmisc.tile([1, 512], DT, tag="dnr")
                            nc.vector.tensor_copy(dnr[:],
                                                  accs[h2][64:65, :])
                            recip = pmisc.tile([1, 512], DT, tag="recip")
                            nc.vector.reciprocal_approx_fast(
                                recip[:], dnr[:])
                            recipb = pmisc.tile([64, 512], DT, tag="recipb")
                            nc.gpsimd.partition_broadcast(recipb[:],
                                                          recip[:])
                            nc.vector.tensor_tensor(
                                attT[qb][pair][h2 * 64:(h2 + 1) * 64, :],
                                accs[h2][0:64, :], recipb[:], MUL)

                def emit_outproj(sb):
                    # last-emitted block's copies split DVE/Act: nothing
                    # else left to overlap, so don't serialize on one engine
                    tail = sb == NSB - 1
                    for stl in range(4):
                        st = sb * 4 + stl
                        ot = pout.tile([128, D], BF, tag="out")
                        for half in range(2):
                            ps = psP.tile([128, 512], DT, tag="pp")
                            for t in range(2):
                                nc.tensor.matmul(
                                    ps[:],
                                    attT[sb][t][:, stl * 128:(stl + 1) * 128],
                                    wot[:, t, half * 512:(half + 1) * 512],
                                    start=(t == 0), stop=(t == 1))
                            dst = ot[:, half * 512:(half + 1) * 512]
                            if tail and half == 1:
                                nc.scalar.activation(dst, ps[:], Act.Copy)
                            else:
                                nc.vector.tensor_copy(dst, ps[:])
                        nc.sync.dma_start(outp[st * 128:(st + 1) * 128, :],
                                          ot[:])

                # Emission order = scheduler priority: each attn leads;
                # proj(sb+1)/outproj(sb-1) emitted after it act as PE filler
                # during the Act-bound softmax stretch. Only valid for
                # causal, where attn(sb) reads K/V blocks <= sb; with a
                # full mask every attn reads ALL K/V blocks, so all
                # projections must be emitted first.
                if causal:
                    emit_proj(0)
                    for sb in range(NSB):
                        emit_attn(sb)
                        if sb + 1 < NSB:
                            emit_proj(sb + 1)
                        if sb >= 1:
                            emit_outproj(sb - 1)
                    emit_outproj(NSB - 1)
                else:
                    for sb in range(NSB):
                        emit_proj(sb)
                    for sb in range(NSB):
                        emit_attn(sb)
                        if sb >= 1:
                            emit_outproj(sb - 1)
                    emit_outproj(NSB - 1)

            UNROLL = 8
            if reps == 1:
                body()
            elif reps < 0:
                # debug: -reps serial bodies, no loop (TimelineSim-able)
                for bi in range(-reps):
                    body(bi)
            elif reps % UNROLL == 0:
                # unrolled: tile tags are shared across the two bodies
                # (natural WAR pipelining) while names stay unique; halves
                # the per-iteration drain + all-engine barrier cost
                with tc.For_i(0, reps // UNROLL, 1):
                    for bi in range(UNROLL):
                        body(bi)
            else:
                with tc.For_i(0, reps, 1):
                    body()

    nc.compile()
    return nc


def _bf16(a):
    import ml_dtypes
    return np.ascontiguousarray(np.asarray(a, np.float32).astype(
        ml_dtypes.bfloat16))


def _consts_array():
    import ml_dtypes
    c = np.zeros((128, NCST), dtype=np.float32)
    c[:, 0:128] = np.triu(np.ones((128, 128), np.float32))
    c[:, 128:NCST] = 1.0
    return c.astype(ml_dtypes.bfloat16)


def make_in_maps(x, mask, Wq, bq, Wk, bk, Wv, bv, Wo, bo):
    x = np.asarray(x, np.float32)
    m = np.asarray(mask)[0, 0]
    mb = (m != 0)
    if mb.all():
        mode = "none"
    elif np.array_equal(mb, np.tril(np.ones((S, S), bool))):
        mode = "causal"
    else:
        mode = "general"
    # bk shifts all scores for a given query equally -> softmax-invariant;
    # only bq and bv affect the output.
    has_bias = bool(np.any(bq) or np.any(bv))

    Wq = np.asarray(Wq, np.float32)
    Wk = np.asarray(Wk, np.float32)
    Wv = np.asarray(Wv, np.float32)
    Wo = np.asarray(Wo, np.float32)
    consts = _consts_array()
    maskt = _bf16(mb.T.astype(np.float32)) if mode == "general" else None

    in_maps = []
    for c in range(NCORES):
        b, hg = divmod(c, HPC)
        cols = slice(hg * DIMS, (hg + 1) * DIMS)
        im = {
            "xt": _bf16(x[b].T),
            "wq": _bf16(Wq[:, cols]),
            "wk": _bf16(Wk[:, cols]),
            "wv": _bf16(Wv[:, cols]),
            "wo": _bf16(Wo[hg * DIMS:(hg + 1) * DIMS, :]),
            "cin": consts,
        }
        if has_bias:
            im["wqb"] = _bf16(np.asarray(bq, np.float32)[None, cols])
            im["wvb"] = _bf16(np.asarray(bv, np.float32)[None, cols])
        if maskt is not None:
            im["maskt"] = maskt
        in_maps.append(im)
    return in_maps, mode, has_bias


def gather_output(results, bo):
    out = np.zeros((B, S, D), dtype=np.float32)
    for c in range(NCORES):
        out[c // HPC] += np.asarray(results[c]["outp"], np.float32)
    out += np.asarray(bo, np.float32)[None, None, :]
    return out


def run(in_maps, mode, has_bias, reps=1, phases=None):
    from concourse.bass_utils import run_bass_kernel_spmd
    key = (mode, has_bias, reps)
    if key not in _PROG_CACHE:
        _PROG_CACHE[key] = build_program(mode, has_bias, reps)
    nc = _PROG_CACHE[key]
    return run_bass_kernel_spmd(nc, in_maps, core_ids=list(range(NCORES)))


def kernel(x, mask, Wq, bq, Wk, bk, Wv, bv, Wo, bo):
    in_maps, mode, has_bias = make_in_maps(x, mask, Wq, bq, Wk, bk, Wv, bv,
                                           Wo, bo)
    r = run(in_maps, mode, has_bias, reps=1)
    return gather_output(r.results, bo)

